# revision 3
# baseline (speedup 1.0000x reference)
"""Trainium2 Bass kernel v2: relational GNN message passing (BlockDecomposition).

v2 changes vs v1 (narrow-H):
  - Slots within each (bucket, src-half, rel) group are sorted by DST ROW
    (then src).  Each (G-tile, rel) incidence becomes one H column whose
    static dst span [j0, j0+W) is the union of the 8 cores' spans,
    W rounded up to a width class in {16,32,48,64,96,128}.
  - H is built packed: per bucket, columns grouped by width class; one
    is_equal + one mult DVE op per (bucket, class).  Total H elements
    drop ~2x (40.7M -> ~20M), which is the v1 bottleneck (DVE 1x mode).
  - Aggregation matmuls write PSUM sub-ranges [j0, j0+W) with
    start=False; each (bucket, rel) chain starts with a cheap K=1
    zero-matmul to clear the accumulator tile.

Numerics: bf16 table/G/H/bw, fp32 PSUM accumulation, fp32 output.
"""

import os
import sys

import numpy as np

# A wedged/degraded device state can slow DMA ~1.6x; a core reset at
# runtime init restores full speed and is otherwise harmless.
os.environ.setdefault("NEURON_RT_RESET_CORES", "1")

sys.path.insert(0, "/opt/trn_rl_repo")

N_NODES = 50000
DIM = 128
N_REL = 8
NCORES = 8
SHARD = N_NODES // NCORES  # 6250
NBUCK = (SHARD + 127) // 128  # 49
PADN = NBUCK * 128  # 6272
LO = 32768

_PAD_ROW = 255.0
CLASSES = (16, 32, 48, 64, 96, 128)

_cache = {}
last_result = None


def _np_dt(dt_name):
    if dt_name == "float32":
        return np.float32
    import ml_dtypes

    return np.dtype(getattr(ml_dtypes, dt_name))


class Layout4:
    """Static packed layout with dst-sorted groups and narrow H columns.

    Columns: one per (bucket, half, tile-in-region, rel) static incidence
    plus one self column per bucket.  Column widths/offsets are data-
    dependent (union of core spans) but shared by all cores, so one
    compiled program serves all 8 cores.
    """

    def __init__(self, gmax, spans):
        # gmax: [NBUCK, N_REL, 2]; spans: dict (b,h,t,r) -> (lo, hi)
        self.gmax = gmax
        self.s = np.zeros((NBUCK, 2, N_REL + 1), dtype=np.int64)
        self.s[:, 0, 1:] = np.cumsum(gmax[:, :, 0], axis=1)
        self.s[:, 1, 1:] = np.cumsum(gmax[:, :, 1], axis=1)
        self.E = self.s[:, :, N_REL]
        self.rt = -(-self.E // 128)  # region tiles
        self.tb = self.rt[:, 0] + self.rt[:, 1] + 1  # + self tile
        self.c0 = np.zeros(NBUCK, dtype=np.int64)
        self.c0[1:] = np.cumsum(self.tb)[:-1]
        self.nt = int(self.tb.sum())
        self.olo = np.zeros(NBUCK, dtype=np.int64)
        self.olo[1:] = np.cumsum(self.rt[:, 0] * 8)[:-1]
        self.ohi = np.zeros(NBUCK, dtype=np.int64)
        self.ohi[1:] = np.cumsum(self.rt[:, 1] * 8)[:-1]
        self.ilo_cols = int((self.rt[:, 0] * 8).sum())
        self.ihi_cols = int((self.rt[:, 1] * 8).sum())

        # enumerate columns per bucket
        # col spec: (class_w, h, t_region, rel, j0, W, off_in_bucket)
        self.cols = []  # [NBUCK] -> list of dict
        self.colmap = {}  # (b, h, t_region, r) -> (bucket, idx in cols[b])
        self.mm = []  # [NBUCK][N_REL+1] -> list of (g_tile, off, j0, W)
        self.classes = []  # [NBUCK] -> list of (w, off_start, ncols)
        self.sumw = np.zeros(NBUCK, dtype=np.int64)
        self.hb0 = np.zeros(NBUCK, dtype=np.int64)
        for b in range(NBUCK):
            raw = []
            for h in range(2):
                ntile = int(self.rt[b, h])
                for t in range(ntile):
                    lo_, hi_ = t * 128, (t + 1) * 128
                    for r in range(N_REL):
                        if self.s[b, h, r] < hi_ and self.s[b, h, r + 1] > lo_:
                            sp = spans.get((b, h, t, r))
                            if sp is None:
                                jlo, jhi = 0, 0
                            else:
                                jlo, jhi = sp
                            W = jhi - jlo + 1
                            for cw in CLASSES:
                                if W <= cw:
                                    W = cw
                                    break
                            j0 = min(jlo, 128 - W)
                            raw.append([W, h, t, r, j0])
            # self column
            raw.append([128, 2, 0, N_REL, 0])
            # sort by class then canonical
            raw.sort(key=lambda x: (x[0], x[1], x[2], x[3]))
            off = 0
            cols_b = []
            classes_b = []
            cls_start, cls_w, cls_n = 0, None, 0
            mm_b = [[] for _ in range(N_REL + 1)]
            for W, h, t, r, j0 in raw:
                if cls_w != W:
                    if cls_n:
                        classes_b.append((cls_w, cls_start, cls_n))
                    cls_w, cls_start, cls_n = W, off, 0
                cls_n += 1
                if r == N_REL:
                    gt = int(self.tb[b]) - 1
                else:
                    gt = (0 if h == 0 else int(self.rt[b, 0])) + t
                mm_b[r].append((gt, off, j0, W))
                self.colmap[(b, h, t, r)] = (b, len(cols_b))
                cols_b.append((W, h, t, r, j0, off))
                off += W
            if cls_n:
                classes_b.append((cls_w, cls_start, cls_n))
            self.cols.append(cols_b)
            self.mm.append(mm_b)
            self.classes.append(classes_b)
            self.sumw[b] = off
        self.hb0[1:] = np.cumsum(self.sumw)[:-1]
        self.sumw_tot = int(self.sumw.sum())
        self.ncols = np.array([len(c) for c in self.cols])
        self.nc0 = np.zeros(NBUCK, dtype=np.int64)
        self.nc0[1:] = np.cumsum(self.ncols)[:-1]
        self.ncols_tot = int(self.ncols.sum())
        # iotaR: per width class, a dense tiled-arange table segment
        maxn = {}
        for b in range(NBUCK):
            for w, off, ncl in self.classes[b]:
                maxn[w] = max(maxn.get(w, 0), ncl)
        self.iotar_off = {}
        off = 0
        for w in sorted(maxn):
            self.iotar_off[w] = off
            off += maxn[w] * w
        self.iotar_cols = off

    def key(self):
        return (
            self.gmax.tobytes(),
            tuple(tuple(c) for b in self.cols for c in b),
        )


def _percore_slots(src, dst, rel, w, sort_mode="dst"):
    """Split messages by dst core.

    sort_mode 'dst': slots ordered by dst row within each group (narrow H
    columns; tile-segment src runs).  'src': pure src order within group
    (v1-style long ascending gather runs; H columns come out full-width).
    """
    percore = []
    cnt = np.zeros((NCORES, NBUCK, N_REL, 2), dtype=np.int64)
    for k in range(NCORES):
        m = (dst >= k * SHARD) & (dst < (k + 1) * SHARD)
        s_k = src[m]
        l_k = dst[m] - k * SHARD
        r_k = rel[m]
        w_k = w[m]
        half = (s_k >= LO).astype(np.int64)
        bucket = l_k // 128
        row = l_k % 128
        if sort_mode == "dst":
            order = np.lexsort((s_k, row, r_k, half, bucket))
        else:
            order = np.lexsort((s_k, r_k, half, bucket))
        s_k, r_k, w_k, half, bucket, row = (
            a[order] for a in (s_k, r_k, w_k, half, bucket, row)
        )
        np.add.at(cnt[k], (bucket, r_k, half), 1)
        percore.append((s_k, l_k, r_k, w_k, half, bucket, row))
    return cnt, percore


def _slot_assign(lay_s, percore_k):
    """Per-core slot positions given static group starts lay_s.

    Slots are assigned to tiles by dst order (narrow H columns), then
    re-sorted by src WITHIN each (group, tile) segment so the gather
    walks ascending HBM addresses (row-buffer locality).
    """
    s_k, l_k, r_k, w_k, half, bucket, row = percore_k
    g = (bucket * 2 + half) * N_REL + r_k
    sizes = np.bincount(g, minlength=NBUCK * 2 * N_REL)
    starts = np.zeros_like(sizes)
    starts[1:] = np.cumsum(sizes)[:-1]
    rank = np.arange(len(g)) - starts[g]  # input order is dst-sorted
    slot = lay_s[bucket, half, r_k] + rank
    tile = slot // 128
    ord2 = np.lexsort((s_k, tile, g))
    rank2 = np.arange(len(g)) - starts[g[ord2]]
    slot2 = np.empty(len(g), dtype=np.int64)
    slot2[ord2] = (
        lay_s[bucket[ord2], half[ord2], r_k[ord2]] + rank2
    )
    tile2 = slot2 // 128
    assert (tile2 == tile).all()
    return slot2, tile2, slot2 % 128


def _prepare_layout(src, dst, rel, w, sort_mode="dst"):
    cnt, percore = _percore_slots(src, dst, rel, w, sort_mode)
    gmax = cnt.max(axis=0)
    # temp starts for span computation
    s = np.zeros((NBUCK, 2, N_REL + 1), dtype=np.int64)
    s[:, 0, 1:] = np.cumsum(gmax[:, :, 0], axis=1)
    s[:, 1, 1:] = np.cumsum(gmax[:, :, 1], axis=1)
    spans = {}
    for k in range(NCORES):
        s_k, l_k, r_k, w_k, half, bucket, row = percore[k]
        slot, tile, p = _slot_assign(s, percore[k])
        # group spans per (b, h, t, r)
        key = ((bucket * 2 + half) * 64 + tile) * 8 + r_k
        order = np.argsort(key, kind="stable")
        ks = key[order]
        rs = row[order]
        boundaries = np.flatnonzero(np.diff(ks)) + 1
        seg_starts = np.concatenate([[0], boundaries])
        seg_ends = np.concatenate([boundaries, [len(ks)]])
        for a, e in zip(seg_starts, seg_ends):
            kk = int(ks[a])
            b = kk // (2 * 64 * 8)
            rem = kk % (2 * 64 * 8)
            h = rem // (64 * 8)
            rem = rem % (64 * 8)
            t = rem // 8
            r = rem % 8
            lo_, hi_ = int(rs[a:e].min()), int(rs[a:e].max())
            cur = spans.get((b, h, t, r))
            if cur is None:
                spans[(b, h, t, r)] = (lo_, hi_)
            else:
                spans[(b, h, t, r)] = (min(cur[0], lo_), max(cur[1], hi_))
    lay = Layout4(gmax, spans)
    return lay, percore


def _prepare_core_meta(lay, percore, keep, dt_name):
    npdt = _np_dt(dt_name)
    # dense col-id map [NBUCK, 2, 64, 8]
    cid_map = np.full((NBUCK, 2, 64, N_REL), -1, dtype=np.int64)
    coff = np.zeros(lay.ncols_tot, dtype=np.int64)  # global col -> off in H
    cj0 = np.zeros(lay.ncols_tot, dtype=np.int64)
    for b in range(NBUCK):
        for i, (W, h, t, r, j0, off) in enumerate(lay.cols[b]):
            gidx = int(lay.nc0[b]) + i
            if r < N_REL:
                cid_map[b, h, t, r] = gidx
            coff[gidx] = off
            cj0[gidx] = j0

    ilo_all, ihi_all, grow_all, gw_all = [], [], [], []
    for k in range(NCORES):
        s_k, l_k, r_k, w_k, half, bucket, row = percore[k]
        slot, tile, p = _slot_assign(lay.s, percore[k])

        # gather idx arrays (position i in region -> col off + i//16, row i%16)
        cols_lo = lay.olo[bucket] + slot // 16
        cols_hi = lay.ohi[bucket] + slot // 16
        prow = slot % 16
        ilo = np.zeros((16, lay.ilo_cols), dtype=np.int16)
        ihi = np.zeros((16, lay.ihi_cols), dtype=np.int16)
        is_lo = half == 0
        ilo[prow[is_lo], cols_lo[is_lo]] = s_k[is_lo].astype(np.int16)
        ihi[prow[~is_lo], cols_hi[~is_lo]] = (s_k[~is_lo] - LO).astype(np.int16)

        # growL / gw: one scalar per (partition, column)
        growL = np.full((128, lay.ncols_tot), _PAD_ROW, dtype=np.float32)
        gwL = np.zeros((128, lay.ncols_tot), dtype=np.float32)
        cid = cid_map[bucket, half, tile, r_k]
        assert (cid >= 0).all()
        growL[p, cid] = row - cj0[cid]
        gwL[p, cid] = w_k
        # self columns: last col of each bucket (class 128, r == N_REL)
        for b in range(NBUCK):
            for i, (W, h, t, r, j0, off) in enumerate(lay.cols[b]):
                if r == N_REL:
                    gidx = int(lay.nc0[b]) + i
                    nrows = min(128, SHARD - b * 128)
                    pr = np.arange(nrows)
                    growL[pr, gidx] = pr
                    gwL[pr, gidx] = keep[
                        k * SHARD + b * 128 : k * SHARD + b * 128 + nrows
                    ]

        ilo_all.append(np.tile(ilo, (8, 1)))
        ihi_all.append(np.tile(ihi, (8, 1)))
        grow_all.append(np.repeat(growL, 2, axis=1).astype(npdt))
        gw_all.append(np.repeat(gwL, 2, axis=1).astype(npdt))
    return ilo_all, ihi_all, grow_all, gw_all


def _build_program(
    dt_name,
    lay,
    repeat=1,
    n_queues=4,
    gbufs=4,
    hbufs=4,
    do_gather=True,
    do_dve=True,
    do_pe=True,
    single_packet=False,
    pair=1,
    dve_pair=True,
    chunk_tiles=16,
    scratch=49152,
):
    from contextlib import ExitStack

    from concourse import bacc, mybir
    import concourse.tile as tile

    DT = getattr(mybir.dt, dt_name)
    f32 = mybir.dt.float32
    i16 = mybir.dt.int16

    nc = bacc.Bacc(
        None,
        target_bir_lowering=False,
        debug=False,
        num_swdge_queues=n_queues,
        dynamic_dma_scratch_size=scratch,
    )

    with tile.TileContext(nc) as tc:
        with tc.tile_pool(name="dram", bufs=1, space="DRAM") as dram:
            xt_d = dram.tile([N_NODES, DIM], DT, kind="ExternalInput", name="xt")
            xown_d = dram.tile([PADN, DIM], DT, kind="ExternalInput", name="xown")
            bw_d = dram.tile([128, (N_REL + 1) * 128], DT, kind="ExternalInput", name="bw")
            iota_d = dram.tile([128, 128], DT, kind="ExternalInput", name="iota")
            iotar_d = dram.tile(
                [128, lay.iotar_cols], DT, kind="ExternalInput", name="iotar"
            )
            ilo_d = dram.tile([128, lay.ilo_cols], i16, kind="ExternalInput", name="ilo")
            ihi_d = dram.tile([128, lay.ihi_cols], i16, kind="ExternalInput", name="ihi")
            grow_d = dram.tile(
                [128, 2 * lay.ncols_tot], DT, kind="ExternalInput", name="grow"
            )
            gw_d = dram.tile(
                [128, 2 * lay.ncols_tot], DT, kind="ExternalInput", name="gw"
            )
            out_d = dram.tile([128, PADN], f32, kind="ExternalOutput", name="outT")

            groups = [
                list(range(p0, min(p0 + pair, NBUCK)))
                for p0 in range(0, NBUCK, pair)
            ]
            max_tb = max(int(lay.tb[bs].sum()) for bs in groups)
            max_sw = int(lay.sumw.max())
            with (
                tc.tile_pool(name="const", bufs=1) as constp,
                tc.tile_pool(name="gpool", bufs=gbufs) as gpool,
                tc.tile_pool(name="hpool", bufs=hbufs) as hpool,
                tc.tile_pool(name="aggsb", bufs=6) as aggsbp,
                tc.tile_pool(name="outsb", bufs=3) as outsbp,
                tc.tile_pool(name="aggps", bufs=4, space="PSUM") as aggpsp,
                tc.tile_pool(name="outps", bufs=2, space="PSUM") as outpsp,
            ):
                iota_s = constp.tile([128, 128], DT)
                iotar_s = constp.tile([128, lay.iotar_cols], DT)
                bw_s = constp.tile([128, (N_REL + 1) * 128], DT)
                grow_s = constp.tile([128, 2 * lay.ncols_tot], DT)
                gw_s = constp.tile([128, 2 * lay.ncols_tot], DT)
                ilo_s = constp.tile([128, lay.ilo_cols], i16)
                ihi_s = constp.tile([128, lay.ihi_cols], i16)
                zc = constp.tile([1, 128], DT)
                nc.sync.dma_start(out=iota_s[:], in_=iota_d[:])
                nc.sync.dma_start(out=iotar_s[:], in_=iotar_d[:])
                nc.sync.dma_start(out=bw_s[:], in_=bw_d[:])
                nc.sync.dma_start(out=grow_s[:], in_=grow_d[:])
                nc.sync.dma_start(out=gw_s[:], in_=gw_d[:])
                nc.sync.dma_start(out=ilo_s[:], in_=ilo_d[:])
                nc.sync.dma_start(out=ihi_s[:], in_=ihi_d[:])
                nc.vector.memset(zc[:], 0.0)

                rep_ctx = ExitStack()
                if repeat > 1:
                    rep_ctx.enter_context(tc.For_i(0, repeat, 1))
                gq = 0
                for bs in groups:
                    nlos = [int(lay.rt[b, 0]) for b in bs]
                    nhis = [int(lay.rt[b, 1]) for b in bs]
                    L = sum(nlos)
                    HT = sum(nhis)
                    G = gpool.tile([128, max_tb, DIM], DT, name="G")
                    if do_gather:
                        olo = int(lay.olo[bs[0]])
                        ohi = int(lay.ohi[bs[0]])
                        t = 0
                        while t < L:
                            t1 = min(t + chunk_tiles, L)
                            nc.gpsimd.dma_gather(
                                G[:, t:t1, :],
                                xt_d[0:LO],
                                ilo_s[:, olo + 8 * t : olo + 8 * t1],
                                (t1 - t) * 128,
                                (t1 - t) * 128,
                                DIM,
                                single_packet=single_packet,
                                queue_num=gq % n_queues,
                            )
                            gq += 1
                            t = t1
                        t = 0
                        while t < HT:
                            t1 = min(t + chunk_tiles, HT)
                            nc.gpsimd.dma_gather(
                                G[:, L + t : L + t1, :],
                                xt_d[LO:N_NODES],
                                ihi_s[:, ohi + 8 * t : ohi + 8 * t1],
                                (t1 - t) * 128,
                                (t1 - t) * 128,
                                DIM,
                                single_packet=single_packet,
                                queue_num=gq % n_queues,
                            )
                            gq += 1
                            t = t1
                        for bi, b in enumerate(bs):
                            nc.sync.dma_start(
                                out=G[:, L + HT + bi, :],
                                in_=xown_d[b * 128 : (b + 1) * 128, :],
                            )

                    for bi, b in enumerate(bs):
                        nb0 = int(lay.nc0[b])
                        nlo = nlos[bi]
                        nhi = nhis[bi]
                        lo_base = sum(nlos[:bi])
                        hi_base = L + sum(nhis[:bi])
                        self_t = L + HT + bi

                        def _gt(gt, nlo=nlo, nhi=nhi, lo_base=lo_base,
                                hi_base=hi_base, self_t=self_t):
                            if gt < nlo:
                                return lo_base + gt
                            if gt < nlo + nhi:
                                return hi_base + (gt - nlo)
                            return self_t

                        H = hpool.tile([128, max_sw], DT, name="H", tag="H")
                        if do_dve:
                            for w_cl, off_cl, ncl in lay.classes[b]:
                                c_start = None
                                for i, (W, h, t, r, j0, off) in enumerate(
                                    lay.cols[b]
                                ):
                                    if off == off_cl:
                                        c_start = nb0 + i
                                        break
                                Hv = H[
                                    :, off_cl : off_cl + ncl * w_cl
                                ].rearrange("p (c w) -> p c w", w=w_cl)
                                if dve_pair:
                                    # all-dense / pair-stride APs: innermost
                                    # step 1 on every operand (2x-mode bid)
                                    Hv4 = H[
                                        :, off_cl : off_cl + ncl * w_cl
                                    ].rearrange(
                                        "p (c u two) -> p c u two", two=2, u=w_cl // 2
                                    )
                                    nc.vector.tensor_tensor(
                                        out=Hv4,
                                        in0=iotar_s[
                                            :,
                                            lay.iotar_off[w_cl] : lay.iotar_off[
                                                w_cl
                                            ]
                                            + ncl * w_cl,
                                        ].rearrange(
                                            "p (c u two) -> p c u two",
                                            two=2,
                                            u=w_cl // 2,
                                        ),
                                        in1=grow_s[
                                            :, 2 * c_start : 2 * (c_start + ncl)
                                        ]
                                        .rearrange("p (c two) -> p c two", two=2)
                                        .unsqueeze(2)
                                        .broadcast_to([128, ncl, w_cl // 2, 2]),
                                        op=mybir.AluOpType.is_equal,
                                    )
                                    nc.vector.tensor_tensor(
                                        out=Hv4,
                                        in0=Hv4,
                                        in1=gw_s[
                                            :, 2 * c_start : 2 * (c_start + ncl)
                                        ]
                                        .rearrange("p (c two) -> p c two", two=2)
                                        .unsqueeze(2)
                                        .broadcast_to([128, ncl, w_cl // 2, 2]),
                                        op=mybir.AluOpType.mult,
                                    )
                                else:
                                    nc.vector.tensor_tensor(
                                        out=Hv,
                                        in0=iota_s[:, 0:w_cl]
                                        .unsqueeze(1)
                                        .broadcast_to([128, ncl, w_cl]),
                                        in1=grow_s[
                                            :, 2 * c_start : 2 * (c_start + ncl)
                                        ]
                                        .rearrange("p (c two) -> p c two", two=2)[
                                            :, :, 0:1
                                        ]
                                        .broadcast_to([128, ncl, w_cl]),
                                        op=mybir.AluOpType.is_equal,
                                    )
                                    nc.vector.tensor_tensor(
                                        out=Hv,
                                        in0=Hv,
                                        in1=gw_s[
                                            :, 2 * c_start : 2 * (c_start + ncl)
                                        ]
                                        .rearrange("p (c two) -> p c two", two=2)[
                                            :, :, 0:1
                                        ]
                                        .broadcast_to([128, ncl, w_cl]),
                                        op=mybir.AluOpType.mult,
                                    )

                        if do_pe:
                            out_ps = outpsp.tile(
                                [128, 128], f32, name="out_ps", space="PSUM"
                            )
                            rels = [
                                r for r in range(N_REL + 1) if lay.mm[b][r]
                            ]
                            for ri, r in enumerate(rels):
                                pairs = lay.mm[b][r]
                                agg_ps = aggpsp.tile(
                                    [128, 128], f32, name="agg_ps", space="PSUM"
                                )
                                full = len(pairs) == 1 and pairs[0][3] == 128
                                if not full:
                                    nc.tensor.matmul(
                                        out=agg_ps[:],
                                        lhsT=zc[:],
                                        rhs=zc[:],
                                        start=True,
                                        stop=False,
                                    )
                                for j, (gt, off, j0, W) in enumerate(pairs):
                                    nc.tensor.matmul(
                                        out=agg_ps[:, j0 : j0 + W],
                                        lhsT=G[:, _gt(gt), :],
                                        rhs=H[:, off : off + W],
                                        start=full,
                                        stop=(j == len(pairs) - 1),
                                    )
                                agg_sb = aggsbp.tile(
                                    [128, 128], DT, name="agg_sb"
                                )
                                nc.scalar.copy(out=agg_sb[:], in_=agg_ps[:])
                                nc.tensor.matmul(
                                    out=out_ps[:],
                                    lhsT=bw_s[:, r * 128 : (r + 1) * 128],
                                    rhs=agg_sb[:],
                                    start=(ri == 0),
                                    stop=(ri == len(rels) - 1),
                                )
                            out_sb = outsbp.tile(
                                [128, 128], f32, name="out_sb"
                            )
                            nc.scalar.copy(out=out_sb[:], in_=out_ps[:])
                            nc.sync.dma_start(
                                out=out_d[:, b * 128 : (b + 1) * 128],
                                in_=out_sb[:],
                            )
                if not do_pe:
                    z = outsbp.tile([128, 128], f32, name="zz")
                    nc.vector.memset(z[:], 0.0)
                    nc.sync.dma_start(out=out_d[:, 0:128], in_=z[:])
                rep_ctx.close()

    nc.compile()
    names = {
        "xt": xt_d.tensor.name,
        "xown": xown_d.tensor.name,
        "bw": bw_d.tensor.name,
        "iota": iota_d.tensor.name,
        "iotar": iotar_d.tensor.name,
        "ilo": ilo_d.tensor.name,
        "ihi": ihi_d.tensor.name,
        "grow": grow_d.tensor.name,
        "gw": gw_d.tensor.name,
        "out": out_d.tensor.name,
    }
    return nc, names


def _block_diag_bw(blocks, dt_name):
    npdt = _np_dt(dt_name)
    nrel1, nb, bs, _ = blocks.shape
    bw = np.zeros((128, nrel1 * 128), dtype=np.float32)
    for r in range(nrel1):
        for a in range(nb):
            bw[a * bs : (a + 1) * bs, r * 128 + a * bs : r * 128 + (a + 1) * bs] = blocks[r, a]
    return bw.astype(npdt)


def _prep(
    x,
    blocks,
    node_keep_mask,
    source,
    target,
    edge_type,
    edge_weights,
    _dt,
    sort_mode="dst",
):
    x = np.asarray(x, dtype=np.float32)
    blocks = np.asarray(blocks, dtype=np.float32)
    keep = np.asarray(node_keep_mask).astype(np.float32)
    source = np.asarray(source).astype(np.int64)
    target = np.asarray(target).astype(np.int64)
    edge_type = np.asarray(edge_type).astype(np.int64)
    edge_weights = np.asarray(edge_weights, dtype=np.float32)

    npdt = _np_dt(_dt)
    src = np.concatenate([source, target])
    dst = np.concatenate([target, source])
    rel = np.concatenate([edge_type, edge_type])
    w = np.concatenate([edge_weights, edge_weights])

    lay, percore = _prepare_layout(src, dst, rel, w, sort_mode)
    ilo_all, ihi_all, grow_all, gw_all = _prepare_core_meta(
        lay, percore, keep, _dt
    )

    xt = x.astype(npdt)
    xown_all = []
    for k in range(NCORES):
        xo = np.zeros((PADN, DIM), dtype=np.float32)
        xo[:SHARD] = x[k * SHARD : (k + 1) * SHARD]
        xown_all.append(xo.astype(npdt))
    bw = _block_diag_bw(blocks, _dt)
    iota = np.tile(np.arange(128, dtype=np.float32), (128, 1)).astype(npdt)
    iotar = np.zeros((128, lay.iotar_cols), dtype=np.float32)
    offs = sorted(lay.iotar_off.items(), key=lambda kv: kv[1])
    for i, (w, off) in enumerate(offs):
        end = offs[i + 1][1] if i + 1 < len(offs) else lay.iotar_cols
        n = (end - off) // w
        iotar[:, off:end] = np.tile(np.arange(w, dtype=np.float32), (128, n))
    iotar = iotar.astype(npdt)
    data = (xt, xown_all, bw, iota, iotar, ilo_all, ihi_all, grow_all, gw_all)
    return lay, data


def _in_maps(names, data):
    xt, xown_all, bw, iota, iotar, ilo_all, ihi_all, grow_all, gw_all = data
    return [
        {
            names["xt"]: xt,
            names["xown"]: xown_all[k],
            names["bw"]: bw,
            names["iota"]: iota,
            names["iotar"]: iotar,
            names["ilo"]: ilo_all[k],
            names["ihi"]: ihi_all[k],
            names["grow"]: grow_all[k],
            names["gw"]: gw_all[k],
        }
        for k in range(NCORES)
    ]


def _get_program(_dt, lay, repeat=1):
    key = (_dt,) + (lay.key(),) + (repeat,)
    if key not in _cache:
        _cache[key] = _build_program(_dt, lay, repeat)
    return _cache[key]


def kernel(x, blocks, node_keep_mask, source, target, edge_type, edge_weights, _dt="bfloat16"):
    from concourse.bass_utils import run_bass_kernel_spmd

    lay, data = _prep(
        x, blocks, node_keep_mask, source, target, edge_type, edge_weights, _dt
    )
    nc, names = _get_program(_dt, lay)
    global last_result
    out = None
    for _attempt in range(3):
        res = run_bass_kernel_spmd(nc, _in_maps(names, data), list(range(NCORES)))
        last_result = res
        out = np.concatenate(
            [
                np.asarray(res.results[k][names["out"]]).T[:SHARD]
                for k in range(NCORES)
            ],
            axis=0,
        ).astype(np.float32)
        if np.isfinite(out).all():
            break
    return out


class _ResidentRunner:
    """Compile once, hold device-resident inputs, run many times."""

    def __init__(self, nc, in_maps, n_cores=8):
        import jax
        import numpy as _np
        from jax.sharding import Mesh, PartitionSpec, NamedSharding
        from jax.experimental.shard_map import shard_map
        from concourse import bass2jax, mybir
        from concourse.bass2jax import _bass_exec_p, install_neuronx_cc_hook

        install_neuronx_cc_hook()
        self.jax = jax
        partition_name = (
            nc.partition_id_tensor.name if nc.partition_id_tensor else None
        )
        in_names, out_names, out_avals = [], [], []
        for alloc in nc.m.functions[0].allocations:
            if not isinstance(alloc, mybir.MemoryLocationSet):
                continue
            name = alloc.memorylocations[0].name
            if alloc.kind == "ExternalInput":
                if name != partition_name:
                    in_names.append(name)
            elif alloc.kind == "ExternalOutput":
                out_names.append(name)
                out_avals.append(
                    jax.core.ShapedArray(
                        tuple(alloc.tensor_shape), mybir.dt.np(alloc.dtype)
                    )
                )
        n_params = len(in_names)
        n_outs = len(out_avals)
        all_in_names = list(in_names) + list(out_names)
        if partition_name is not None:
            all_in_names.append(partition_name)
        self.out_names = out_names
        self.out_avals = out_avals
        self.n_cores = n_cores

        def _body(*args):
            operands = list(args)
            if partition_name is not None:
                operands.append(bass2jax.partition_id_tensor())
            return tuple(
                _bass_exec_p.bind(
                    *operands,
                    out_avals=tuple(out_avals),
                    in_names=tuple(all_in_names),
                    out_names=tuple(out_names),
                    lowering_input_output_aliases=(),
                    sim_require_finite=True,
                    sim_require_nnan=True,
                    nc=nc,
                )
            )

        devices = jax.devices()[:n_cores]
        mesh = Mesh(_np.asarray(devices), ("core",))
        self.fn = jax.jit(
            shard_map(
                _body,
                mesh=mesh,
                in_specs=(PartitionSpec("core"),) * (n_params + n_outs),
                out_specs=(PartitionSpec("core"),) * n_outs,
                check_rep=False,
            ),
            keep_unused=True,
        )
        sharding = NamedSharding(mesh, PartitionSpec("core"))
        concat_in = [
            _np.concatenate(
                [_np.asarray(in_maps[c][name]) for c in range(n_cores)], axis=0
            )
            for name in in_names
        ]
        concat_zero = [
            _np.zeros((n_cores * a.shape[0], *a.shape[1:]), a.dtype)
            for a in out_avals
        ]
        self.dev_in = [jax.device_put(a, sharding) for a in concat_in]
        self.dev_zero = [jax.device_put(a, sharding) for a in concat_zero]

    def run(self):
        outs = self.fn(*self.dev_in, *self.dev_zero)
        self.jax.block_until_ready(outs)
        return outs

    def results(self):
        outs = self.run()
        res = []
        for c in range(self.n_cores):
            res.append(
                {
                    name: np.asarray(outs[i]).reshape(
                        self.n_cores, *self.out_avals[i].shape
                    )[c]
                    for i, name in enumerate(self.out_names)
                }
            )
        return res

    def time_ns(self, n_warm=2, n_runs=7):
        import time

        for _ in range(n_warm):
            self.run()
        best = float("inf")
        for _ in range(n_runs):
            t0 = time.perf_counter()
            self.run()
            best = min(best, time.perf_counter() - t0)
        return best * 1e9


def measure_hw_ns(inputs, _dt="bfloat16", big_rep=257, n_runs=7):
    ResidentRunner = _ResidentRunner

    lay, data = _prep(_dt=_dt, **inputs)
    walls = {}
    out_big = None
    for rep in (1, big_rep):
        nc, names = _get_program(_dt, lay, rep)
        maps = _in_maps(names, data)
        r = ResidentRunner(nc, maps, NCORES)
        walls[rep] = r.time_ns(n_warm=2, n_runs=n_runs)
        if rep == big_rep:
            res = r.results()
            out_big = np.concatenate(
                [np.asarray(res[k][names["out"]]).T[:SHARD] for k in range(NCORES)],
                axis=0,
            ).astype(np.float32)
    body_ns = (walls[big_rep] - walls[1]) / (big_rep - 1)
    print(
        f"wall rep=1: {walls[1] / 1e6:.1f} ms, rep={big_rep}: "
        f"{walls[big_rep] / 1e6:.1f} ms -> body {body_ns:.0f} ns"
    )
    return body_ns, out_big


# revision 4
# speedup vs baseline: 1.2105x; 1.2105x over previous
"""Trainium2 Bass kernel v2: relational GNN message passing (BlockDecomposition).

v2 changes vs v1 (narrow-H):
  - Slots within each (bucket, src-half, rel) group are sorted by DST ROW
    (then src).  Each (G-tile, rel) incidence becomes one H column whose
    static dst span [j0, j0+W) is the union of the 8 cores' spans,
    W rounded up to a width class in {16,32,48,64,96,128}.
  - H is built packed: per bucket, columns grouped by width class; one
    is_equal + one mult DVE op per (bucket, class).  Total H elements
    drop ~2x (40.7M -> ~20M), which is the v1 bottleneck (DVE 1x mode).
  - Aggregation matmuls write PSUM sub-ranges [j0, j0+W) with
    start=False; each (bucket, rel) chain starts with a cheap K=1
    zero-matmul to clear the accumulator tile.

Numerics: bf16 table/G/H/bw, fp32 PSUM accumulation, fp32 output.
"""

import os
import sys

import numpy as np

# A wedged/degraded device state can slow DMA ~1.6x; a core reset at
# runtime init restores full speed and is otherwise harmless.
os.environ.setdefault("NEURON_RT_RESET_CORES", "1")

sys.path.insert(0, "/opt/trn_rl_repo")

N_NODES = 50000
DIM = 128
N_REL = 8
NCORES = 8
SHARD = N_NODES // NCORES  # 6250
NBUCK = (SHARD + 127) // 128  # 49
PADN = NBUCK * 128  # 6272
LO = 32768

_PAD_ROW = 255.0
CLASSES = (16, 32, 48, 64, 96, 128)

_cache = {}
last_result = None


def _np_dt(dt_name):
    if dt_name == "float32":
        return np.float32
    import ml_dtypes

    return np.dtype(getattr(ml_dtypes, dt_name))


class Layout4:
    """Static packed layout with dst-sorted groups and narrow H columns.

    Columns: one per (bucket, half, tile-in-region, rel) static incidence
    plus one self column per bucket.  Column widths/offsets are data-
    dependent (union of core spans) but shared by all cores, so one
    compiled program serves all 8 cores.
    """

    def __init__(self, gmax, spans):
        # gmax: [NBUCK, N_REL, 2]; spans: dict (b,h,t,r) -> (lo, hi)
        self.gmax = gmax
        self.s = np.zeros((NBUCK, 2, N_REL + 1), dtype=np.int64)
        self.s[:, 0, 1:] = np.cumsum(gmax[:, :, 0], axis=1)
        self.s[:, 1, 1:] = np.cumsum(gmax[:, :, 1], axis=1)
        self.E = self.s[:, :, N_REL]
        self.rt = -(-self.E // 128)  # region tiles
        self.tb = self.rt[:, 0] + self.rt[:, 1] + 1  # + self tile
        self.c0 = np.zeros(NBUCK, dtype=np.int64)
        self.c0[1:] = np.cumsum(self.tb)[:-1]
        self.nt = int(self.tb.sum())
        self.olo = np.zeros(NBUCK, dtype=np.int64)
        self.olo[1:] = np.cumsum(self.rt[:, 0] * 8)[:-1]
        self.ohi = np.zeros(NBUCK, dtype=np.int64)
        self.ohi[1:] = np.cumsum(self.rt[:, 1] * 8)[:-1]
        self.ilo_cols = int((self.rt[:, 0] * 8).sum())
        self.ihi_cols = int((self.rt[:, 1] * 8).sum())

        # enumerate columns per bucket
        # col spec: (class_w, h, t_region, rel, j0, W, off_in_bucket)
        self.cols = []  # [NBUCK] -> list of dict
        self.colmap = {}  # (b, h, t_region, r) -> (bucket, idx in cols[b])
        self.mm = []  # [NBUCK][N_REL+1] -> list of (g_tile, off, j0, W)
        self.classes = []  # [NBUCK] -> list of (w, off_start, ncols)
        self.sumw = np.zeros(NBUCK, dtype=np.int64)
        self.hb0 = np.zeros(NBUCK, dtype=np.int64)
        for b in range(NBUCK):
            raw = []
            for h in range(2):
                ntile = int(self.rt[b, h])
                for t in range(ntile):
                    lo_, hi_ = t * 128, (t + 1) * 128
                    for r in range(N_REL):
                        if self.s[b, h, r] < hi_ and self.s[b, h, r + 1] > lo_:
                            sp = spans.get((b, h, t, r))
                            if sp is None:
                                jlo, jhi = 0, 0
                            else:
                                jlo, jhi = sp
                            W = jhi - jlo + 1
                            for cw in CLASSES:
                                if W <= cw:
                                    W = cw
                                    break
                            j0 = min(jlo, 128 - W)
                            raw.append([W, h, t, r, j0])
            # self column
            raw.append([128, 2, 0, N_REL, 0])
            # sort by class then canonical
            raw.sort(key=lambda x: (x[0], x[1], x[2], x[3]))
            off = 0
            cols_b = []
            classes_b = []
            cls_start, cls_w, cls_n = 0, None, 0
            mm_b = [[] for _ in range(N_REL + 1)]
            for W, h, t, r, j0 in raw:
                if cls_w != W:
                    if cls_n:
                        classes_b.append((cls_w, cls_start, cls_n))
                    cls_w, cls_start, cls_n = W, off, 0
                cls_n += 1
                if r == N_REL:
                    gt = int(self.tb[b]) - 1
                else:
                    gt = (0 if h == 0 else int(self.rt[b, 0])) + t
                mm_b[r].append((gt, off, j0, W))
                self.colmap[(b, h, t, r)] = (b, len(cols_b))
                cols_b.append((W, h, t, r, j0, off))
                off += W
            if cls_n:
                classes_b.append((cls_w, cls_start, cls_n))
            self.cols.append(cols_b)
            self.mm.append(mm_b)
            self.classes.append(classes_b)
            self.sumw[b] = off
        self.hb0[1:] = np.cumsum(self.sumw)[:-1]
        self.sumw_tot = int(self.sumw.sum())
        self.ncols = np.array([len(c) for c in self.cols])
        self.nc0 = np.zeros(NBUCK, dtype=np.int64)
        self.nc0[1:] = np.cumsum(self.ncols)[:-1]
        self.ncols_tot = int(self.ncols.sum())
        # iotaR: per width class, a dense tiled-arange table segment
        maxn = {}
        for b in range(NBUCK):
            for w, off, ncl in self.classes[b]:
                maxn[w] = max(maxn.get(w, 0), ncl)
        self.iotar_off = {}
        off = 0
        for w in sorted(maxn):
            self.iotar_off[w] = off
            off += maxn[w] * w
        self.iotar_cols = off

    def key(self):
        return (
            self.gmax.tobytes(),
            tuple(tuple(c) for b in self.cols for c in b),
        )


def _percore_slots(src, dst, rel, w, sort_mode="dst"):
    """Split messages by dst core.

    sort_mode 'dst': slots ordered by dst row within each group (narrow H
    columns; tile-segment src runs).  'src': pure src order within group
    (v1-style long ascending gather runs; H columns come out full-width).
    """
    percore = []
    cnt = np.zeros((NCORES, NBUCK, N_REL, 2), dtype=np.int64)
    for k in range(NCORES):
        m = (dst >= k * SHARD) & (dst < (k + 1) * SHARD)
        s_k = src[m]
        l_k = dst[m] - k * SHARD
        r_k = rel[m]
        w_k = w[m]
        half = (s_k >= LO).astype(np.int64)
        bucket = l_k // 128
        row = l_k % 128
        if sort_mode == "dst":
            order = np.lexsort((s_k, row, r_k, half, bucket))
        else:
            order = np.lexsort((s_k, r_k, half, bucket))
        s_k, r_k, w_k, half, bucket, row = (
            a[order] for a in (s_k, r_k, w_k, half, bucket, row)
        )
        np.add.at(cnt[k], (bucket, r_k, half), 1)
        percore.append((s_k, l_k, r_k, w_k, half, bucket, row))
    return cnt, percore


def _slot_assign(lay_s, percore_k):
    """Per-core slot positions given static group starts lay_s.

    Slots are assigned to tiles by dst order (narrow H columns), then
    re-sorted by src WITHIN each (group, tile) segment so the gather
    walks ascending HBM addresses (row-buffer locality).
    """
    s_k, l_k, r_k, w_k, half, bucket, row = percore_k
    g = (bucket * 2 + half) * N_REL + r_k
    sizes = np.bincount(g, minlength=NBUCK * 2 * N_REL)
    starts = np.zeros_like(sizes)
    starts[1:] = np.cumsum(sizes)[:-1]
    rank = np.arange(len(g)) - starts[g]  # input order is dst-sorted
    slot = lay_s[bucket, half, r_k] + rank
    tile = slot // 128
    ord2 = np.lexsort((s_k, tile, g))
    rank2 = np.arange(len(g)) - starts[g[ord2]]
    slot2 = np.empty(len(g), dtype=np.int64)
    slot2[ord2] = (
        lay_s[bucket[ord2], half[ord2], r_k[ord2]] + rank2
    )
    tile2 = slot2 // 128
    assert (tile2 == tile).all()
    return slot2, tile2, slot2 % 128


def _prepare_layout(src, dst, rel, w, sort_mode="dst"):
    cnt, percore = _percore_slots(src, dst, rel, w, sort_mode)
    gmax = cnt.max(axis=0)
    # temp starts for span computation
    s = np.zeros((NBUCK, 2, N_REL + 1), dtype=np.int64)
    s[:, 0, 1:] = np.cumsum(gmax[:, :, 0], axis=1)
    s[:, 1, 1:] = np.cumsum(gmax[:, :, 1], axis=1)
    spans = {}
    for k in range(NCORES):
        s_k, l_k, r_k, w_k, half, bucket, row = percore[k]
        slot, tile, p = _slot_assign(s, percore[k])
        # group spans per (b, h, t, r)
        key = ((bucket * 2 + half) * 64 + tile) * 8 + r_k
        order = np.argsort(key, kind="stable")
        ks = key[order]
        rs = row[order]
        boundaries = np.flatnonzero(np.diff(ks)) + 1
        seg_starts = np.concatenate([[0], boundaries])
        seg_ends = np.concatenate([boundaries, [len(ks)]])
        for a, e in zip(seg_starts, seg_ends):
            kk = int(ks[a])
            b = kk // (2 * 64 * 8)
            rem = kk % (2 * 64 * 8)
            h = rem // (64 * 8)
            rem = rem % (64 * 8)
            t = rem // 8
            r = rem % 8
            lo_, hi_ = int(rs[a:e].min()), int(rs[a:e].max())
            cur = spans.get((b, h, t, r))
            if cur is None:
                spans[(b, h, t, r)] = (lo_, hi_)
            else:
                spans[(b, h, t, r)] = (min(cur[0], lo_), max(cur[1], hi_))
    lay = Layout4(gmax, spans)
    return lay, percore


def _prepare_core_meta(lay, percore, keep, dt_name):
    npdt = _np_dt(dt_name)
    # dense col-id map [NBUCK, 2, 64, 8]
    cid_map = np.full((NBUCK, 2, 64, N_REL), -1, dtype=np.int64)
    coff = np.zeros(lay.ncols_tot, dtype=np.int64)  # global col -> off in H
    cj0 = np.zeros(lay.ncols_tot, dtype=np.int64)
    for b in range(NBUCK):
        for i, (W, h, t, r, j0, off) in enumerate(lay.cols[b]):
            gidx = int(lay.nc0[b]) + i
            if r < N_REL:
                cid_map[b, h, t, r] = gidx
            coff[gidx] = off
            cj0[gidx] = j0

    ilo_all, ihi_all, grow_all, gw_all = [], [], [], []
    for k in range(NCORES):
        s_k, l_k, r_k, w_k, half, bucket, row = percore[k]
        slot, tile, p = _slot_assign(lay.s, percore[k])

        # gather idx arrays (position i in region -> col off + i//16, row
        # i%16).  Pad slots forward-fill the previous real index so the
        # HBM stream stays monotone within a region instead of spiking
        # to row 0 at every group tail.
        is_lo = half == 0
        flo = np.full(lay.ilo_cols * 16, -1, dtype=np.int64)
        fhi = np.full(lay.ihi_cols * 16, -1, dtype=np.int64)
        flo[(lay.olo[bucket] * 16 + slot)[is_lo]] = s_k[is_lo]
        fhi[(lay.ohi[bucket] * 16 + slot)[~is_lo]] = s_k[~is_lo] - LO
        for f in (flo, fhi):
            pad = f < 0
            idxs = np.where(~pad, np.arange(len(f)), 0)
            np.maximum.accumulate(idxs, out=idxs)
            f[:] = f[idxs]
            f[f < 0] = 0  # leading pads before any real slot
        ilo = flo.reshape(lay.ilo_cols, 16).T.astype(np.int16).copy()
        ihi = fhi.reshape(lay.ihi_cols, 16).T.astype(np.int16).copy()

        # growL / gw: one scalar per (partition, column)
        growL = np.full((128, lay.ncols_tot), _PAD_ROW, dtype=np.float32)
        gwL = np.zeros((128, lay.ncols_tot), dtype=np.float32)
        cid = cid_map[bucket, half, tile, r_k]
        assert (cid >= 0).all()
        growL[p, cid] = row - cj0[cid]
        gwL[p, cid] = w_k
        # self columns: last col of each bucket (class 128, r == N_REL)
        for b in range(NBUCK):
            for i, (W, h, t, r, j0, off) in enumerate(lay.cols[b]):
                if r == N_REL:
                    gidx = int(lay.nc0[b]) + i
                    nrows = min(128, SHARD - b * 128)
                    pr = np.arange(nrows)
                    growL[pr, gidx] = pr
                    gwL[pr, gidx] = keep[
                        k * SHARD + b * 128 : k * SHARD + b * 128 + nrows
                    ]

        ilo_all.append(np.tile(ilo, (8, 1)))
        ihi_all.append(np.tile(ihi, (8, 1)))
        grow_all.append(np.repeat(growL, 2, axis=1).astype(npdt))
        gw_all.append(np.repeat(gwL, 2, axis=1).astype(npdt))
    return ilo_all, ihi_all, grow_all, gw_all


def _build_program(
    dt_name,
    lay,
    repeat=1,
    n_queues=4,
    gbufs=4,
    hbufs=4,
    do_gather=True,
    do_dve=True,
    do_pe=True,
    single_packet=False,
    pair=1,
    dve_pair=True,
    chunk_tiles=16,
    scratch=49152,
):
    from contextlib import ExitStack

    from concourse import bacc, mybir
    import concourse.tile as tile

    DT = getattr(mybir.dt, dt_name)
    f32 = mybir.dt.float32
    i16 = mybir.dt.int16

    nc = bacc.Bacc(
        None,
        target_bir_lowering=False,
        debug=False,
        num_swdge_queues=n_queues,
        dynamic_dma_scratch_size=scratch,
    )

    with tile.TileContext(nc) as tc:
        with tc.tile_pool(name="dram", bufs=1, space="DRAM") as dram:
            xt_d = dram.tile([N_NODES, DIM], DT, kind="ExternalInput", name="xt")
            xown_d = dram.tile([PADN, DIM], DT, kind="ExternalInput", name="xown")
            bw_d = dram.tile([128, (N_REL + 1) * 128], DT, kind="ExternalInput", name="bw")
            iota_d = dram.tile([128, 128], DT, kind="ExternalInput", name="iota")
            iotar_d = dram.tile(
                [128, lay.iotar_cols], DT, kind="ExternalInput", name="iotar"
            )
            ilo_d = dram.tile([128, lay.ilo_cols], i16, kind="ExternalInput", name="ilo")
            ihi_d = dram.tile([128, lay.ihi_cols], i16, kind="ExternalInput", name="ihi")
            grow_d = dram.tile(
                [128, 2 * lay.ncols_tot], DT, kind="ExternalInput", name="grow"
            )
            gw_d = dram.tile(
                [128, 2 * lay.ncols_tot], DT, kind="ExternalInput", name="gw"
            )
            out_d = dram.tile([128, PADN], f32, kind="ExternalOutput", name="outT")

            groups = [
                list(range(p0, min(p0 + pair, NBUCK)))
                for p0 in range(0, NBUCK, pair)
            ]
            max_tb = max(int(lay.tb[bs].sum()) for bs in groups)
            max_sw = int(lay.sumw.max())
            with (
                tc.tile_pool(name="const", bufs=1) as constp,
                tc.tile_pool(name="gpool", bufs=gbufs) as gpool,
                tc.tile_pool(name="hpool", bufs=hbufs) as hpool,
                tc.tile_pool(name="aggsb", bufs=6) as aggsbp,
                tc.tile_pool(name="outsb", bufs=3) as outsbp,
                tc.tile_pool(name="aggps", bufs=4, space="PSUM") as aggpsp,
                tc.tile_pool(name="outps", bufs=2, space="PSUM") as outpsp,
            ):
                iota_s = constp.tile([128, 128], DT)
                iotar_s = constp.tile([128, lay.iotar_cols], DT)
                bw_s = constp.tile([128, (N_REL + 1) * 128], DT)
                grow_s = constp.tile([128, 2 * lay.ncols_tot], DT)
                gw_s = constp.tile([128, 2 * lay.ncols_tot], DT)
                ilo_s = constp.tile([128, lay.ilo_cols], i16)
                ihi_s = constp.tile([128, lay.ihi_cols], i16)
                zc = constp.tile([1, 128], DT)
                nc.sync.dma_start(out=iota_s[:], in_=iota_d[:])
                nc.sync.dma_start(out=iotar_s[:], in_=iotar_d[:])
                nc.sync.dma_start(out=bw_s[:], in_=bw_d[:])
                nc.sync.dma_start(out=grow_s[:], in_=grow_d[:])
                nc.sync.dma_start(out=gw_s[:], in_=gw_d[:])
                nc.sync.dma_start(out=ilo_s[:], in_=ilo_d[:])
                nc.sync.dma_start(out=ihi_s[:], in_=ihi_d[:])
                nc.vector.memset(zc[:], 0.0)

                rep_ctx = ExitStack()
                if repeat > 1:
                    rep_ctx.enter_context(tc.For_i(0, repeat, 1))
                gq = 0
                for bs in groups:
                    nlos = [int(lay.rt[b, 0]) for b in bs]
                    nhis = [int(lay.rt[b, 1]) for b in bs]
                    L = sum(nlos)
                    HT = sum(nhis)
                    G = gpool.tile([128, max_tb, DIM], DT, name="G")
                    if do_gather:
                        olo = int(lay.olo[bs[0]])
                        ohi = int(lay.ohi[bs[0]])
                        t = 0
                        while t < L:
                            t1 = min(t + chunk_tiles, L)
                            nc.gpsimd.dma_gather(
                                G[:, t:t1, :],
                                xt_d[0:LO],
                                ilo_s[:, olo + 8 * t : olo + 8 * t1],
                                (t1 - t) * 128,
                                (t1 - t) * 128,
                                DIM,
                                single_packet=single_packet,
                                queue_num=gq % n_queues,
                            )
                            gq += 1
                            t = t1
                        t = 0
                        while t < HT:
                            t1 = min(t + chunk_tiles, HT)
                            nc.gpsimd.dma_gather(
                                G[:, L + t : L + t1, :],
                                xt_d[LO:N_NODES],
                                ihi_s[:, ohi + 8 * t : ohi + 8 * t1],
                                (t1 - t) * 128,
                                (t1 - t) * 128,
                                DIM,
                                single_packet=single_packet,
                                queue_num=gq % n_queues,
                            )
                            gq += 1
                            t = t1
                        for bi, b in enumerate(bs):
                            nc.sync.dma_start(
                                out=G[:, L + HT + bi, :],
                                in_=xown_d[b * 128 : (b + 1) * 128, :],
                            )

                    for bi, b in enumerate(bs):
                        nb0 = int(lay.nc0[b])
                        nlo = nlos[bi]
                        nhi = nhis[bi]
                        lo_base = sum(nlos[:bi])
                        hi_base = L + sum(nhis[:bi])
                        self_t = L + HT + bi

                        def _gt(gt, nlo=nlo, nhi=nhi, lo_base=lo_base,
                                hi_base=hi_base, self_t=self_t):
                            if gt < nlo:
                                return lo_base + gt
                            if gt < nlo + nhi:
                                return hi_base + (gt - nlo)
                            return self_t

                        H = hpool.tile([128, max_sw], DT, name="H", tag="H")
                        if do_dve:
                            for w_cl, off_cl, ncl in lay.classes[b]:
                                c_start = None
                                for i, (W, h, t, r, j0, off) in enumerate(
                                    lay.cols[b]
                                ):
                                    if off == off_cl:
                                        c_start = nb0 + i
                                        break
                                Hv = H[
                                    :, off_cl : off_cl + ncl * w_cl
                                ].rearrange("p (c w) -> p c w", w=w_cl)
                                if dve_pair:
                                    # all-dense / pair-stride APs: innermost
                                    # step 1 on every operand (2x-mode bid)
                                    Hv4 = H[
                                        :, off_cl : off_cl + ncl * w_cl
                                    ].rearrange(
                                        "p (c u two) -> p c u two", two=2, u=w_cl // 2
                                    )
                                    nc.vector.tensor_tensor(
                                        out=Hv4,
                                        in0=iotar_s[
                                            :,
                                            lay.iotar_off[w_cl] : lay.iotar_off[
                                                w_cl
                                            ]
                                            + ncl * w_cl,
                                        ].rearrange(
                                            "p (c u two) -> p c u two",
                                            two=2,
                                            u=w_cl // 2,
                                        ),
                                        in1=grow_s[
                                            :, 2 * c_start : 2 * (c_start + ncl)
                                        ]
                                        .rearrange("p (c two) -> p c two", two=2)
                                        .unsqueeze(2)
                                        .broadcast_to([128, ncl, w_cl // 2, 2]),
                                        op=mybir.AluOpType.is_equal,
                                    )
                                    nc.vector.tensor_tensor(
                                        out=Hv4,
                                        in0=Hv4,
                                        in1=gw_s[
                                            :, 2 * c_start : 2 * (c_start + ncl)
                                        ]
                                        .rearrange("p (c two) -> p c two", two=2)
                                        .unsqueeze(2)
                                        .broadcast_to([128, ncl, w_cl // 2, 2]),
                                        op=mybir.AluOpType.mult,
                                    )
                                else:
                                    nc.vector.tensor_tensor(
                                        out=Hv,
                                        in0=iota_s[:, 0:w_cl]
                                        .unsqueeze(1)
                                        .broadcast_to([128, ncl, w_cl]),
                                        in1=grow_s[
                                            :, 2 * c_start : 2 * (c_start + ncl)
                                        ]
                                        .rearrange("p (c two) -> p c two", two=2)[
                                            :, :, 0:1
                                        ]
                                        .broadcast_to([128, ncl, w_cl]),
                                        op=mybir.AluOpType.is_equal,
                                    )
                                    nc.vector.tensor_tensor(
                                        out=Hv,
                                        in0=Hv,
                                        in1=gw_s[
                                            :, 2 * c_start : 2 * (c_start + ncl)
                                        ]
                                        .rearrange("p (c two) -> p c two", two=2)[
                                            :, :, 0:1
                                        ]
                                        .broadcast_to([128, ncl, w_cl]),
                                        op=mybir.AluOpType.mult,
                                    )

                        if do_pe:
                            out_ps = outpsp.tile(
                                [128, 128], f32, name="out_ps", space="PSUM"
                            )
                            rels = [
                                r for r in range(N_REL + 1) if lay.mm[b][r]
                            ]
                            for ri, r in enumerate(rels):
                                pairs = lay.mm[b][r]
                                agg_ps = aggpsp.tile(
                                    [128, 128], f32, name="agg_ps", space="PSUM"
                                )
                                full = len(pairs) == 1 and pairs[0][3] == 128
                                if not full:
                                    nc.tensor.matmul(
                                        out=agg_ps[:],
                                        lhsT=zc[:],
                                        rhs=zc[:],
                                        start=True,
                                        stop=False,
                                    )
                                for j, (gt, off, j0, W) in enumerate(pairs):
                                    nc.tensor.matmul(
                                        out=agg_ps[:, j0 : j0 + W],
                                        lhsT=G[:, _gt(gt), :],
                                        rhs=H[:, off : off + W],
                                        start=full,
                                        stop=(j == len(pairs) - 1),
                                    )
                                agg_sb = aggsbp.tile(
                                    [128, 128], DT, name="agg_sb"
                                )
                                nc.scalar.copy(out=agg_sb[:], in_=agg_ps[:])
                                nc.tensor.matmul(
                                    out=out_ps[:],
                                    lhsT=bw_s[:, r * 128 : (r + 1) * 128],
                                    rhs=agg_sb[:],
                                    start=(ri == 0),
                                    stop=(ri == len(rels) - 1),
                                )
                            out_sb = outsbp.tile(
                                [128, 128], f32, name="out_sb"
                            )
                            nc.scalar.copy(out=out_sb[:], in_=out_ps[:])
                            nc.sync.dma_start(
                                out=out_d[:, b * 128 : (b + 1) * 128],
                                in_=out_sb[:],
                            )
                if not do_pe:
                    z = outsbp.tile([128, 128], f32, name="zz")
                    nc.vector.memset(z[:], 0.0)
                    nc.sync.dma_start(out=out_d[:, 0:128], in_=z[:])
                rep_ctx.close()

    nc.compile()
    names = {
        "xt": xt_d.tensor.name,
        "xown": xown_d.tensor.name,
        "bw": bw_d.tensor.name,
        "iota": iota_d.tensor.name,
        "iotar": iotar_d.tensor.name,
        "ilo": ilo_d.tensor.name,
        "ihi": ihi_d.tensor.name,
        "grow": grow_d.tensor.name,
        "gw": gw_d.tensor.name,
        "out": out_d.tensor.name,
    }
    return nc, names


def _block_diag_bw(blocks, dt_name):
    npdt = _np_dt(dt_name)
    nrel1, nb, bs, _ = blocks.shape
    bw = np.zeros((128, nrel1 * 128), dtype=np.float32)
    for r in range(nrel1):
        for a in range(nb):
            bw[a * bs : (a + 1) * bs, r * 128 + a * bs : r * 128 + (a + 1) * bs] = blocks[r, a]
    return bw.astype(npdt)


def _prep(
    x,
    blocks,
    node_keep_mask,
    source,
    target,
    edge_type,
    edge_weights,
    _dt,
    sort_mode="dst",
):
    x = np.asarray(x, dtype=np.float32)
    blocks = np.asarray(blocks, dtype=np.float32)
    keep = np.asarray(node_keep_mask).astype(np.float32)
    source = np.asarray(source).astype(np.int64)
    target = np.asarray(target).astype(np.int64)
    edge_type = np.asarray(edge_type).astype(np.int64)
    edge_weights = np.asarray(edge_weights, dtype=np.float32)

    npdt = _np_dt(_dt)
    src = np.concatenate([source, target])
    dst = np.concatenate([target, source])
    rel = np.concatenate([edge_type, edge_type])
    w = np.concatenate([edge_weights, edge_weights])

    lay, percore = _prepare_layout(src, dst, rel, w, sort_mode)
    ilo_all, ihi_all, grow_all, gw_all = _prepare_core_meta(
        lay, percore, keep, _dt
    )

    xt = x.astype(npdt)
    xown_all = []
    for k in range(NCORES):
        xo = np.zeros((PADN, DIM), dtype=np.float32)
        xo[:SHARD] = x[k * SHARD : (k + 1) * SHARD]
        xown_all.append(xo.astype(npdt))
    bw = _block_diag_bw(blocks, _dt)
    iota = np.tile(np.arange(128, dtype=np.float32), (128, 1)).astype(npdt)
    iotar = np.zeros((128, lay.iotar_cols), dtype=np.float32)
    offs = sorted(lay.iotar_off.items(), key=lambda kv: kv[1])
    for i, (w, off) in enumerate(offs):
        end = offs[i + 1][1] if i + 1 < len(offs) else lay.iotar_cols
        n = (end - off) // w
        iotar[:, off:end] = np.tile(np.arange(w, dtype=np.float32), (128, n))
    iotar = iotar.astype(npdt)
    data = (xt, xown_all, bw, iota, iotar, ilo_all, ihi_all, grow_all, gw_all)
    return lay, data


def _in_maps(names, data):
    xt, xown_all, bw, iota, iotar, ilo_all, ihi_all, grow_all, gw_all = data
    return [
        {
            names["xt"]: xt,
            names["xown"]: xown_all[k],
            names["bw"]: bw,
            names["iota"]: iota,
            names["iotar"]: iotar,
            names["ilo"]: ilo_all[k],
            names["ihi"]: ihi_all[k],
            names["grow"]: grow_all[k],
            names["gw"]: gw_all[k],
        }
        for k in range(NCORES)
    ]


def _get_program(_dt, lay, repeat=1):
    key = (_dt,) + (lay.key(),) + (repeat,)
    if key not in _cache:
        _cache[key] = _build_program(_dt, lay, repeat)
    return _cache[key]


def kernel(x, blocks, node_keep_mask, source, target, edge_type, edge_weights, _dt="bfloat16"):
    from concourse.bass_utils import run_bass_kernel_spmd

    lay, data = _prep(
        x, blocks, node_keep_mask, source, target, edge_type, edge_weights, _dt
    )
    nc, names = _get_program(_dt, lay)
    global last_result
    out = None
    for _attempt in range(3):
        res = run_bass_kernel_spmd(nc, _in_maps(names, data), list(range(NCORES)))
        last_result = res
        out = np.concatenate(
            [
                np.asarray(res.results[k][names["out"]]).T[:SHARD]
                for k in range(NCORES)
            ],
            axis=0,
        ).astype(np.float32)
        if np.isfinite(out).all():
            break
    return out


class _ResidentRunner:
    """Compile once, hold device-resident inputs, run many times."""

    def __init__(self, nc, in_maps, n_cores=8):
        import jax
        import numpy as _np
        from jax.sharding import Mesh, PartitionSpec, NamedSharding
        from jax.experimental.shard_map import shard_map
        from concourse import bass2jax, mybir
        from concourse.bass2jax import _bass_exec_p, install_neuronx_cc_hook

        install_neuronx_cc_hook()
        self.jax = jax
        partition_name = (
            nc.partition_id_tensor.name if nc.partition_id_tensor else None
        )
        in_names, out_names, out_avals = [], [], []
        for alloc in nc.m.functions[0].allocations:
            if not isinstance(alloc, mybir.MemoryLocationSet):
                continue
            name = alloc.memorylocations[0].name
            if alloc.kind == "ExternalInput":
                if name != partition_name:
                    in_names.append(name)
            elif alloc.kind == "ExternalOutput":
                out_names.append(name)
                out_avals.append(
                    jax.core.ShapedArray(
                        tuple(alloc.tensor_shape), mybir.dt.np(alloc.dtype)
                    )
                )
        n_params = len(in_names)
        n_outs = len(out_avals)
        all_in_names = list(in_names) + list(out_names)
        if partition_name is not None:
            all_in_names.append(partition_name)
        self.out_names = out_names
        self.out_avals = out_avals
        self.n_cores = n_cores

        def _body(*args):
            operands = list(args)
            if partition_name is not None:
                operands.append(bass2jax.partition_id_tensor())
            return tuple(
                _bass_exec_p.bind(
                    *operands,
                    out_avals=tuple(out_avals),
                    in_names=tuple(all_in_names),
                    out_names=tuple(out_names),
                    lowering_input_output_aliases=(),
                    sim_require_finite=True,
                    sim_require_nnan=True,
                    nc=nc,
                )
            )

        devices = jax.devices()[:n_cores]
        mesh = Mesh(_np.asarray(devices), ("core",))
        self.fn = jax.jit(
            shard_map(
                _body,
                mesh=mesh,
                in_specs=(PartitionSpec("core"),) * (n_params + n_outs),
                out_specs=(PartitionSpec("core"),) * n_outs,
                check_rep=False,
            ),
            keep_unused=True,
        )
        sharding = NamedSharding(mesh, PartitionSpec("core"))
        concat_in = [
            _np.concatenate(
                [_np.asarray(in_maps[c][name]) for c in range(n_cores)], axis=0
            )
            for name in in_names
        ]
        concat_zero = [
            _np.zeros((n_cores * a.shape[0], *a.shape[1:]), a.dtype)
            for a in out_avals
        ]
        self.dev_in = [jax.device_put(a, sharding) for a in concat_in]
        self.dev_zero = [jax.device_put(a, sharding) for a in concat_zero]

    def run(self):
        outs = self.fn(*self.dev_in, *self.dev_zero)
        self.jax.block_until_ready(outs)
        return outs

    def results(self):
        outs = self.run()
        res = []
        for c in range(self.n_cores):
            res.append(
                {
                    name: np.asarray(outs[i]).reshape(
                        self.n_cores, *self.out_avals[i].shape
                    )[c]
                    for i, name in enumerate(self.out_names)
                }
            )
        return res

    def time_ns(self, n_warm=2, n_runs=7):
        import time

        for _ in range(n_warm):
            self.run()
        best = float("inf")
        for _ in range(n_runs):
            t0 = time.perf_counter()
            self.run()
            best = min(best, time.perf_counter() - t0)
        return best * 1e9


def measure_hw_ns(inputs, _dt="bfloat16", big_rep=257, n_runs=7):
    ResidentRunner = _ResidentRunner

    lay, data = _prep(_dt=_dt, **inputs)
    walls = {}
    out_big = None
    for rep in (1, big_rep):
        nc, names = _get_program(_dt, lay, rep)
        maps = _in_maps(names, data)
        r = ResidentRunner(nc, maps, NCORES)
        walls[rep] = r.time_ns(n_warm=2, n_runs=n_runs)
        if rep == big_rep:
            res = r.results()
            out_big = np.concatenate(
                [np.asarray(res[k][names["out"]]).T[:SHARD] for k in range(NCORES)],
                axis=0,
            ).astype(np.float32)
    body_ns = (walls[big_rep] - walls[1]) / (big_rep - 1)
    print(
        f"wall rep=1: {walls[1] / 1e6:.1f} ms, rep={big_rep}: "
        f"{walls[big_rep] / 1e6:.1f} ms -> body {body_ns:.0f} ns"
    )
    return body_ns, out_big
